# revision 1
# baseline (speedup 1.0000x reference)
"""Causal multi-head attention for Trainium2, 8-core tensor-parallel over heads.

Problem: B=4, S=2048, D=1024, H=16 heads (dk=64), fp32.
    q = x @ w_q.T ; k = x @ w_k.T ; v = x @ w_v.T   (per-head split)
    out = softmax(causal(q k^T / 8)) v, concat heads, @ w_o.T + b_o

Sharding: core c owns heads (2c, 2c+1) = channels [128c, 128c+128).
Each core computes q/k/v projections for its two heads over the full batch,
runs causal attention, and produces a partial output projection
outT_c = (w_o[:, ch_c] a_c^T) of shape [1024, B*S]; the host sums the 8
partials, transposes, and adds b_o.

Per-core dataflow (all matmuls in fp32r = full PE rate, ~1e-4 rel err):
  - x is pre-transposed on host to xT [B, D, S] so the contraction dim D
    lands on SBUF partitions.
  - qT, kT [128ch, S] per batch via wT-stationary matmuls (heads stacked:
    h0 on partitions 0-63, h1 on 64-127).
  - v produced transposed like q/k then PE-transposed to natural [tok, ch]
    blocks, stored as [v_h | ones64] stationaries: the AV matmul
    out = [v|1]^T P then yields both the attention output (rows 0-63) and
    the softmax denominator replicated on rows 64-127 — no partition
    reduction needed anywhere.
  - scores are computed transposed (keys on partitions): sT = kT^T qT via
    row-packed matmuls (two heads concurrently in row groups 0-1/2-3).
  - softmax without max-subtraction (scores are ~N(0,1); exp is safe in
    fp32), causal mask only on diagonal 128x128 blocks via a precomputed
    additive -1e9 mask; fully-masked column ranges are memset to -1e9.
  - normalization: reciprocal of the replicated denominator rows then one
    elementwise multiply, written straight into the stacked aT layout that
    the output projection consumes.
"""

import numpy as np

import concourse.bass as bass
import concourse.tile as tile
from concourse import mybir
from concourse import bass_utils

f32 = mybir.dt.float32
f32r = mybir.dt.float32r
u32 = mybir.dt.uint32
AF = mybir.ActivationFunctionType

B, S, D, H = 4, 2048, 1024, 16
DK = D // H            # 64
NCORES = 8
PT = 128               # partition tile
CHUNK = 512            # query chunk (fp32 matmul max moving dim)
NEG = -1.0e9

_DMA_CLASSES = {"InstDMACopy", "InstTriggeredCopy", "InstDMATranspose", "InstDMAGatherAnt"}


def _split_multi_waits(nc):
    """This walrus build allows at most one sync-wait per TPB instruction;
    hoist extra waits onto single-wait NoOps on the same engine."""
    n = 0
    for f in nc.m.functions:
        for blk in f.blocks:
            new = []
            for inst in blk.instructions:
                si = inst.sync_info
                if si is not None and si.on_wait and len(si.on_wait) > 1:
                    ws = list(si.on_wait)
                    for w in ws[:-1]:
                        new.append(mybir.InstNoOp(
                            name=f"I-wfix-{n}", ins=[], outs=[], engine=inst.engine,
                            sync_info=mybir.SyncInfo(on_wait=[w], on_update=[])))
                        n += 1
                    inst.sync_info = mybir.SyncInfo(
                        on_wait=[ws[-1]], on_update=list(si.on_update))
                new.append(inst)
            blk.instructions = new
    return n


def build(Bc=B, Sc=S, x_bufs=8, split_waits=True, q_bufs=2, v_bufs=2, a_bufs=2,
          p_bufs=3, vt_bufs=2, os_bufs=6, dm_bufs=4, mm_bufs=2, sc_bufs=2,
          av_bufs=2, do_attn=True, do_outproj=True):
    """Build the per-core Bass program. Same program for all 8 cores; only
    the weight data differs per core."""
    from contextlib import ExitStack

    KT = D // PT           # 8 contraction tiles
    NCH = Sc // CHUNK      # query chunks per batch
    NTT = Sc // PT         # token/key tiles per batch

    nc = bass.Bass("TRN2", target_bir_lowering=False, debug=False)

    xT_d = nc.dram_tensor("xT", [Bc, D, Sc], f32, kind="ExternalInput")
    wqT_d = nc.dram_tensor("wqT", [D, PT], f32, kind="ExternalInput")
    wkT_d = nc.dram_tensor("wkT", [D, PT], f32, kind="ExternalInput")
    wvT_d = nc.dram_tensor("wvT", [D, PT], f32, kind="ExternalInput")
    woT_d = nc.dram_tensor("woT", [PT, D], f32, kind="ExternalInput")
    id_d = nc.dram_tensor("ident", [PT, PT], f32, kind="ExternalInput")
    mask_d = nc.dram_tensor("mask", [PT, PT], f32, kind="ExternalInput")
    out_d = nc.dram_tensor("outT", [D, Bc * Sc], f32, kind="ExternalOutput")

    with tile.TileContext(nc) as tc, ExitStack() as ctx:
        singles = ctx.enter_context(tc.tile_pool(name="singles", bufs=1))
        pool_x = ctx.enter_context(tc.tile_pool(name="x", bufs=x_bufs))
        pool_q = ctx.enter_context(tc.tile_pool(name="q", bufs=q_bufs * NCH))
        pool_k = ctx.enter_context(tc.tile_pool(name="k", bufs=q_bufs * NCH))
        pool_v = ctx.enter_context(tc.tile_pool(name="v", bufs=v_bufs * NTT))
        pool_a = ctx.enter_context(tc.tile_pool(name="a", bufs=2 * a_bufs))
        pool_vt = ctx.enter_context(tc.tile_pool(name="vt", bufs=vt_bufs))
        pool_p = ctx.enter_context(tc.tile_pool(name="p", bufs=p_bufs))
        pool_dm = ctx.enter_context(tc.tile_pool(name="dm", bufs=dm_bufs))
        pool_os = ctx.enter_context(tc.tile_pool(name="os", bufs=os_bufs))
        ps_mm = ctx.enter_context(tc.tile_pool(name="psmm", bufs=mm_bufs, space="PSUM"))
        ps_sc = ctx.enter_context(tc.tile_pool(name="pssc", bufs=sc_bufs, space="PSUM"))
        ps_av = ctx.enter_context(tc.tile_pool(name="psav", bufs=av_bufs, space="PSUM"))

        # ---- constants ----
        wq_sb = singles.tile([PT, KT, PT], f32r)
        wk_sb = singles.tile([PT, KT, PT], f32r)
        wv_sb = singles.tile([PT, KT, PT], f32r)
        for wsb, wd in ((wq_sb, wqT_d), (wk_sb, wkT_d), (wv_sb, wvT_d)):
            nc.sync.dma_start(
                out=wsb[:, :, :],
                in_=wd.ap().bitcast(f32r).rearrange("(kt p) c -> p kt c", p=PT))
        wo_sb = singles.tile([PT, D], f32r)
        nc.sync.dma_start(out=wo_sb[:, :], in_=woT_d.ap().bitcast(f32r))
        id_sb = singles.tile([PT, PT], f32)
        nc.sync.dma_start(out=id_sb[:, :], in_=id_d.ap())
        mask_sb = singles.tile([PT, PT], f32)
        nc.sync.dma_start(out=mask_sb[:, :], in_=mask_d.ap())

        for b in range(Bc):
            # ---- load xT tiles for this batch ----
            x_sb = []
            for kt in range(KT):
                xt = pool_x.tile([PT, Sc], f32r, tag="x")
                nc.sync.dma_start(out=xt[:, :],
                                  in_=xT_d.ap()[b, kt * PT:(kt + 1) * PT, :].bitcast(f32r))
                x_sb.append(xt)

            # ---- projections (per-chunk tiles for fine-grained deps) ----
            qTs, kTs, v_tiles = [], [], []
            for c in range(NCH):
                cw = slice(c * CHUNK, (c + 1) * CHUNK)
                qc = pool_q.tile([PT, CHUNK], f32r, tag="qT", name=f"q{c}")
                kc = pool_k.tile([PT, CHUNK], f32r, tag="kT", name=f"k{c}")
                qTs.append(qc)
                kTs.append(kc)
                for wsb, dst in ((wq_sb, qc), (wk_sb, kc)):
                    ps = ps_mm.tile([PT, CHUNK], f32, tag="mm")
                    for kt in range(KT):
                        nc.tensor.matmul(ps[:, :], wsb[:, kt, :], x_sb[kt][:, cw],
                                         start=(kt == 0), stop=(kt == KT - 1))
                    nc.scalar.copy(dst[:, :], ps[:, :])
                # v: transposed projection then PE-transpose to natural
                psv = ps_mm.tile([PT, CHUNK], f32, tag="mm")
                for kt in range(KT):
                    nc.tensor.matmul(psv[:, :], wv_sb[:, kt, :], x_sb[kt][:, cw],
                                     start=(kt == 0), stop=(kt == KT - 1))
                vt = pool_vt.tile([PT, CHUNK], f32, tag="vt")
                nc.scalar.copy(vt[:, :], psv[:, :])
                pst = ps_mm.tile([PT, CHUNK], f32, tag="mm")
                for j in range(CHUNK // PT):
                    nc.tensor.transpose(pst[:, j * PT:(j + 1) * PT],
                                        vt[:, j * PT:(j + 1) * PT], id_sb[:, :])
                for j in range(CHUNK // PT):
                    t = c * (CHUNK // PT) + j
                    vtile = pool_v.tile([PT, 256], f32r, tag="v", name=f"v{t}")
                    v_tiles.append(vtile)
                    nc.gpsimd.memset(
                        vtile[:, :].rearrange("p (g x) -> p g x", x=128)
                        [:, :, DK:128].bitcast(u32), 0x3F800000)
                    src = pst[:, j * PT:(j + 1) * PT].rearrange(
                        "p (g x) -> p g x", x=DK)           # [128, 2, 64]
                    dst = vtile[:, :].rearrange(
                        "p (g x) -> p g x", x=128)[:, :, 0:DK]
                    nc.vector.tensor_copy(dst, src)

            # ---- attention, chunk by chunk ----
            aTs = []
            for c in range(NCH if do_attn else 0):
                cw = slice(c * CHUNK, (c + 1) * CHUNK)
                nkt = (c + 1) * (CHUNK // PT)      # causal: key tiles 0..nkt-1
                pso = {}
                for h in (0, 1):
                    pso[h] = ps_av.tile([PT, CHUNK], f32, tag="av", name=f"pso{h}")
                for kt0 in range(0, nkt, 2):
                    for h in (0, 1):
                        hp = slice(h * DK, (h + 1) * DK)
                        pss = ps_sc.tile([PT, 2 * CHUNK], f32, tag="sc")
                        for d in (0, 1):
                            kt = kt0 + d
                            nc.tensor.matmul(
                                pss[:, d * CHUNK:(d + 1) * CHUNK],
                                kTs[kt // (CHUNK // PT)]
                                [hp, (kt % (CHUNK // PT)) * PT:
                                 (kt % (CHUNK // PT) + 1) * PT],
                                qTs[c][hp, :],
                                start=True, stop=True)
                        # causal handling on diagonal key tiles: triangular
                        # additive mask on the partial 128x128 block (DVE);
                        # fully-masked leading columns are never exp'd — the
                        # P region is pre-zeroed on gpsimd off the critical
                        # path and exp covers only the valid column ranges.
                        P = pool_p.tile([PT, 2 * CHUNK], f32r, tag="P")
                        i0 = kt0 - (c * (CHUNK // PT))
                        diag = i0 >= 0
                        if diag:
                            for d, i in ((0, i0), (1, i0 + 1)):
                                if i > 0:
                                    nc.gpsimd.memset(
                                        P[:, d * CHUNK: d * CHUNK + i * PT]
                                        .bitcast(u32), 0)
                            for d, i in ((0, i0), (1, i0 + 1)):
                                off = d * CHUNK
                                nc.vector.tensor_add(
                                    pss[:, off + i * PT: off + (i + 1) * PT],
                                    pss[:, off + i * PT: off + (i + 1) * PT],
                                    mask_sb[:, :])
                                nc.scalar.activation(
                                    out=P[:, off + i * PT:(d + 1) * CHUNK],
                                    in_=pss[:, off + i * PT:(d + 1) * CHUNK],
                                    func=AF.Exp)
                        else:
                            nc.scalar.activation(out=P[:, :], in_=pss[:, :],
                                                 func=AF.Exp)
                        for d in (0, 1):
                            kt = kt0 + d
                            nc.tensor.matmul(
                                pso[h][:, :],
                                v_tiles[kt][:, h * 128:(h + 1) * 128],
                                P[:, d * CHUNK:(d + 1) * CHUNK],
                                start=(kt == 0), stop=(kt == nkt - 1),
                                skip_group_check=True)
                # normalize into the per-chunk stacked aT
                aTc = pool_a.tile([PT, CHUNK], f32r, tag="aT", name=f"aT{c}")
                aTs.append(aTc)
                for h in (0, 1):
                    dm = pool_dm.tile([DK, CHUNK], f32, tag="dm")
                    nc.vector.reciprocal(dm[:, :], pso[h][DK:2 * DK, :])
                    nc.vector.tensor_mul(aTc[h * DK:(h + 1) * DK, :],
                                         pso[h][0:DK, :], dm[:, :])

            # ---- output projection (partial, transposed) ----
            for c in range(NCH if (do_attn and do_outproj) else 0):
                for n in range(D // PT):
                    psp = ps_mm.tile([PT, CHUNK], f32, tag="mm")
                    nc.tensor.matmul(psp[:, :], wo_sb[:, n * PT:(n + 1) * PT],
                                     aTs[c][:, :], start=True, stop=True)
                    ost = pool_os.tile([PT, CHUNK], f32, tag="os")
                    nc.vector.tensor_copy(ost[:, :], psp[:, :])
                    nc.sync.dma_start(
                        out=out_d.ap()[n * PT:(n + 1) * PT,
                                       b * Sc + c * CHUNK:
                                       b * Sc + (c + 1) * CHUNK],
                        in_=ost[:, :])

    if split_waits:
        _split_multi_waits(nc)
    return nc


_build_cache = {}


def _get_program(Bc=B, Sc=S):
    key = (Bc, Sc)
    if key not in _build_cache:
        _build_cache[key] = build(Bc, Sc)
    return _build_cache[key]


def make_in_maps(x, w_q, w_k, w_v, w_o):
    """Host-side sharding: returns per-core input dicts."""
    Bc, Sc, Dc = x.shape
    xT = np.ascontiguousarray(x.transpose(0, 2, 1)).astype(np.float32)
    ident = np.eye(PT, dtype=np.float32)
    jj, qq = np.meshgrid(np.arange(PT), np.arange(PT), indexing="ij")
    mask = np.where(jj <= qq, 0.0, NEG).astype(np.float32)
    scale = DK ** -0.5
    in_maps = []
    for c in range(NCORES):
        rows = slice(PT * c, PT * (c + 1))
        in_maps.append({
            "xT": xT,
            "wqT": np.ascontiguousarray((w_q[rows, :] * scale).T).astype(np.float32),
            "wkT": np.ascontiguousarray(w_k[rows, :].T).astype(np.float32),
            "wvT": np.ascontiguousarray(w_v[rows, :].T).astype(np.float32),
            "woT": np.ascontiguousarray(w_o[:, rows].T).astype(np.float32),
            "ident": ident,
            "mask": mask,
        })
    return in_maps


def run_on_hw(in_maps, Bc=B, Sc=S, trace=False):
    nc = _get_program(Bc, Sc)
    return bass_utils.run_bass_kernel_spmd(
        nc, in_maps, core_ids=list(range(NCORES)), trace=trace)


def kernel(x, w_q, w_k, w_v, w_o, b_o):
    x = np.asarray(x, dtype=np.float32)
    w_q = np.asarray(w_q, dtype=np.float32)
    w_k = np.asarray(w_k, dtype=np.float32)
    w_v = np.asarray(w_v, dtype=np.float32)
    w_o = np.asarray(w_o, dtype=np.float32)
    b_o = np.asarray(b_o, dtype=np.float32)
    Bc, Sc, Dc = x.shape
    in_maps = make_in_maps(x, w_q, w_k, w_v, w_o)
    res = run_on_hw(in_maps, Bc, Sc)
    outT = np.zeros((D, Bc * Sc), dtype=np.float32)
    for c in range(NCORES):
        outT += res.results[c]["outT"]
    out = outT.T.reshape(Bc, Sc, D) + b_o
    return out.astype(np.float32)



# revision 43
# speedup vs baseline: 1.9021x; 1.9021x over previous
"""Causal multi-head attention for Trainium2, 8-core batch x head-group parallel.

Problem: B=4, S=2048, D=1024, H=16 heads (dk=64), fp32 in/out.
    q = x @ w_q.T ; k = x @ w_k.T ; v = x @ w_v.T   (per-head split)
    out = softmax(causal(q k^T / 8)) v, concat heads, @ w_o.T + b_o

Sharding: core c owns batch (c % 4) and head group g = c // 4 (heads 8g..8g+7,
i.e. channels [512g, 512g+512)).  Each core computes q/k/v projections for its
512 channels over its single batch, runs causal attention for its 8 heads, and
produces the partial output projection outTp = (w_o[:, ch] a^T) of shape
[1024, S] in bf16; the host sums the two partials per batch (f32), transposes,
and adds b_o.

All device compute is bf16 (inputs converted host-side), fp32 PSUM accumulate:
rel err vs the fp32 reference lands ~1e-3, well under the 2e-2 gate, and bf16
halves DMA/SBUF footprint and DVE copy cost vs the fp32r baseline.

Per-core dataflow:
  - x pre-transposed on host to xT [D, S] so the contraction dim D lands on
    SBUF partitions; loaded once (8 tiles [128, S]).
  - qT, kT [512ch, S] via wT-stationary matmuls; channel tile g' holds head
    pair (2g', 2g'+1) stacked on partitions 0-63 / 64-127, which row-packs the
    dk=64 score matmuls onto disjoint PE row halves.
  - v computed in natural [tok, ch] layout (xT-chunk-stationary, wvT moving,
    N=512) -- no PE transposes needed -- and stored per token tile as
    [128, 8 heads x (64 v | 64 ones)]: the AV matmul out = [v|1]^T P then
    yields both the attention output (rows 0-63) and the softmax denominator
    replicated on rows 64-127 -- no partition reduction anywhere.
  - scores computed transposed (keys on partitions): sT = kT^T qT per 128-key
    tile, fp32 PSUM; softmax without max-subtraction (scores ~N(0,1), exp is
    safe in fp32), causal mask only on diagonal 128x128 blocks via a
    precomputed additive -1e9 mask; fully-masked column ranges are memset on
    gpsimd off the critical path, and the score/AV matmuls are narrowed to the
    valid query range on diagonal tiles.
  - normalization: reciprocal of the replicated denominator rows then one
    elementwise multiply, written straight into the stacked aT layout (bf16)
    that the output projection consumes.
"""

import numpy as np
import ml_dtypes

import concourse.bass as bass
import concourse.tile as tile
from concourse import mybir
from concourse import bass_utils

f32 = mybir.dt.float32
bf16 = mybir.dt.bfloat16
u32 = mybir.dt.uint32
AF = mybir.ActivationFunctionType
np_bf16 = ml_dtypes.bfloat16

B, S, D, H = 4, 2048, 1024, 16
DK = D // H            # 64
HC = H // 2            # 8 heads per core
CH = HC * DK           # 512 channels per core
NCORES = 8
PT = 128               # partition tile
CHUNK = 512            # query chunk
NEG = -1.0e9

_DMA_CLASSES = {"InstDMACopy", "InstTriggeredCopy", "InstDMATranspose", "InstDMAGatherAnt"}


def _split_multi_waits(nc):
    """This walrus build allows at most one sync-wait per TPB instruction;
    hoist extra waits onto single-wait NoOps on the same engine."""
    n = 0
    for f in nc.m.functions:
        for blk in f.blocks:
            new = []
            for inst in blk.instructions:
                si = inst.sync_info
                if si is not None and si.on_wait and len(si.on_wait) > 1:
                    ws = list(si.on_wait)
                    for w in ws[:-1]:
                        new.append(mybir.InstNoOp(
                            name=f"I-wfix-{n}", ins=[], outs=[], engine=inst.engine,
                            sync_info=mybir.SyncInfo(on_wait=[w], on_update=[])))
                        n += 1
                    inst.sync_info = mybir.SyncInfo(
                        on_wait=[ws[-1]], on_update=list(si.on_update))
                new.append(inst)
            blk.instructions = new
    return n


def build(Sc=S, split_waits=True, p_bufs=8, a_bufs=17, os_bufs=6, dm_bufs=3,
          mm_bufs=2, sc_bufs=4, av_bufs=2):
    """Build the per-core Bass program. Same program for all 8 cores; only
    the weight/x data differs per core."""
    from contextlib import ExitStack

    KT = D // PT           # 8 contraction tiles
    NCH = Sc // CHUNK      # query chunks
    NTT = Sc // PT         # token/key tiles
    NG = HC // 2           # 4 head-pair groups

    nc = bass.Bass("TRN2", target_bir_lowering=False, debug=False)

    xT_d = nc.dram_tensor("xT", [D, Sc], bf16, kind="ExternalInput")
    w3_d = nc.dram_tensor("w3", [D, 3 * CH], bf16, kind="ExternalInput")
    # ramp pack: group-0 q/k columns + all v columns, duplicated from w3 so
    # the first attention block's weights arrive in one small early transfer
    wr_d = nc.dram_tensor("wr", [D, 2 * PT], bf16, kind="ExternalInput")
    wo_d = nc.dram_tensor("wo", [CH, D], bf16, kind="ExternalInput")
    mask_d = nc.dram_tensor("mask", [PT, PT], f32, kind="ExternalInput")
    out_d = nc.dram_tensor("outTp", [D, Sc], bf16, kind="ExternalOutput")

    with tile.TileContext(nc) as tc, ExitStack() as ctx:
        singles = ctx.enter_context(tc.tile_pool(name="singles", bufs=1))
        pool_p = ctx.enter_context(tc.tile_pool(name="p", bufs=p_bufs))
        pool_a = ctx.enter_context(tc.tile_pool(name="a", bufs=a_bufs))
        pool_os = ctx.enter_context(tc.tile_pool(name="os", bufs=os_bufs))
        pool_dm = ctx.enter_context(tc.tile_pool(name="dm", bufs=dm_bufs))
        ps_mm = ctx.enter_context(tc.tile_pool(name="psmm", bufs=mm_bufs, space="PSUM"))
        ps_sc = ctx.enter_context(tc.tile_pool(name="pssc", bufs=sc_bufs, space="PSUM"))
        ps_av = ctx.enter_context(tc.tile_pool(name="psav", bufs=av_bufs, space="PSUM"))

        # ---- constants / persistent tensors ----
        # DMA order matters: x arrives in per-(kt, chunk) tiles and w3 in
        # per-kt qk/v halves so the first q/k projection chunk and the first
        # v tiles land ~7us in, instead of waiting for monolithic transfers;
        # later x chunks, wo, and mask queue up behind.
        # HWDGE issues serialize at ~625ns each, so batch the inputs into a
        # handful of transfers: x per chunk column, w3 in 2-kt slabs
        w3_sb = singles.tile([PT, KT, 3 * CH], bf16)
        wr_sb = singles.tile([PT, KT, 2 * PT], bf16)
        x_sb = [singles.tile([PT, KT, CHUNK], bf16, name=f"xc{c}")
                for c in range(NCH)]
        nc.sync.dma_start(
            out=wr_sb[:, :, :],
            in_=wr_d.ap().rearrange("(kt p) c -> p kt c", p=PT))
        nc.sync.dma_start(
            out=x_sb[0][:, :, :],
            in_=xT_d.ap()[:, 0:CHUNK].rearrange("(kt p) c -> p kt c", p=PT))
        mask_sb = singles.tile([PT, PT], f32)
        nc.sync.dma_start(out=mask_sb[:, :], in_=mask_d.ap())
        for kt in range(0, KT, 2):
            nc.sync.dma_start(
                out=w3_sb[:, kt:kt + 2, :],
                in_=w3_d.ap()[kt * PT:(kt + 2) * PT, :]
                .rearrange("(kt p) c -> p kt c", p=PT))
        for c in range(1, NCH):
            nc.sync.dma_start(
                out=x_sb[c][:, :, :],
                in_=xT_d.ap()[:, c * CHUNK:(c + 1) * CHUNK]
                .rearrange("(kt p) c -> p kt c", p=PT))
        wo_sb = singles.tile([PT, NG, D], bf16)
        nc.sync.dma_start(
            out=wo_sb[:, :, :],
            in_=wo_d.ap().rearrange("(g p) c -> p g c", p=PT))

        # per-chunk q/k tiles for fine-grained deps
        qTs = [[singles.tile([PT, CHUNK], bf16, name=f"q{g}_{c}")
                for c in range(NCH)] for g in range(NG)]
        kTs = [[singles.tile([PT, CHUNK], bf16, name=f"k{g}_{c}")
                for c in range(NCH)] for g in range(NG)]
        # v tiles: per token tile, 8 heads x [64 v-ch | 64 ones]
        v_sb = [singles.tile([PT, HC * 2 * DK], bf16, name=f"v{t}")
                for t in range(NTT)]
        for t in range(NTT):
            # pre-set the ones blocks (bf16 1.0 pairs = 0x3F803F80)
            nc.gpsimd.memset(
                v_sb[t][:, :].bitcast(u32)
                .rearrange("p (h x) -> p h x", x=DK)[:, :, DK // 2:],
                0x3F803F80)

        def gen_v(t):
            """Generator filler: v projection for token tile t, yielding
            after every 2 matmuls so the pacer can weave ~426ns steps."""
            tc_, tw = t // 4, slice((t % 4) * PT, (t % 4 + 1) * PT)
            psv = ps_mm.tile([PT, CH], f32, tag="mm")
            for kt in range(KT):
                nc.tensor.matmul(psv[:, :], x_sb[tc_][:, kt, tw],
                                 w3_sb[:, kt, 2 * CH:3 * CH],
                                 start=(kt == 0), stop=(kt == KT - 1))
                if kt % 2 == 1:
                    yield 426
            dst = v_sb[t][:, :].rearrange("p (h x) -> p h x", x=2 * DK)[:, :, 0:DK]
            src = psv[:, :].rearrange("p (h x) -> p h x", x=DK)
            nc.vector.tensor_copy(dst, src)

        def gen_qk1(g, c, w_off, dst, ramp=False):
            ps = ps_mm.tile([PT, CHUNK], f32, tag="mm")
            for kt in range(KT):
                w = (wr_sb[:, kt, (w_off // CH) * PT:(w_off // CH + 1) * PT]
                     if ramp else
                     w3_sb[:, kt, w_off + g * PT:w_off + (g + 1) * PT])
                nc.tensor.matmul(
                    ps[:, :], w, x_sb[c][:, kt, :],
                    start=(kt == 0), stop=(kt == KT - 1))
                if kt % 2 == 1:
                    yield 426
            nc.vector.tensor_copy(dst[:, :], ps[:, :])

        def emit_attn(c, g, tick=None, need_v=None):
            nkt = (c + 1) * (CHUNK // PT)      # causal: key tiles 0..nkt-1
            pso = {}
            for h in (0, 1):
                pso[h] = ps_av.tile([PT, CHUNK], f32, tag="av", name=f"pso{h}")
            for kt in range(nkt):
                if need_v is not None:
                    need_v(kt)
                i = kt - (c * (CHUNK // PT))
                q0 = max(i, 0) * PT   # first valid query col
                # the two heads' K=64 score matmuls are emitted back-to-back:
                # they sit on disjoint PE row halves (partitions 0-63/64-127,
                # auto tile_position), so real hardware runs them concurrently
                pss, P = {}, {}
                for h in (0, 1):
                    hp = slice(h * DK, (h + 1) * DK)
                    pss[h] = ps_sc.tile([PT, CHUNK], f32, tag="sc", name="pss")
                    nc.tensor.matmul(
                        pss[h][:, q0:CHUNK],
                        kTs[g][kt // 4][hp, (kt % 4) * PT:(kt % 4 + 1) * PT],
                        qTs[g][c][hp, q0:CHUNK],
                        start=True, stop=True)
                for h in (0, 1):
                    P[h] = pool_p.tile([PT, CHUNK], bf16, tag="P", name="P")
                    if i >= 0:
                        # diagonal key tile: triangular additive mask on the
                        # partial 128x128 block; fully-masked leading columns
                        # are never exp'd -- pre-zeroed on gpsimd.
                        if i > 0:
                            nc.gpsimd.memset(P[h][:, 0:q0].bitcast(u32), 0)
                        nc.vector.tensor_add(
                            pss[h][:, q0:q0 + PT], pss[h][:, q0:q0 + PT],
                            mask_sb[:, :])
                    nc.scalar.activation(out=P[h][:, q0:CHUNK],
                                         in_=pss[h][:, q0:CHUNK], func=AF.Exp)
                for h in (0, 1):
                    hh = 2 * g + h
                    nc.tensor.matmul(
                        pso[h][:, q0:CHUNK],
                        v_sb[kt][:, hh * 2 * DK:(hh + 1) * 2 * DK],
                        P[h][:, q0:CHUNK],
                        start=(kt == 0), stop=(kt == nkt - 1),
                        skip_group_check=True)
                    if tick is not None:
                        # ACT exp outpaces this iteration's PE work; let the
                        # pacer slot a low-priority PE filler group here.
                        tick(act_ns=62 + (CHUNK - q0) * 1.075,
                             pe_ns=2 * (CHUNK - q0) * 0.4167)
            # normalize into the per-(chunk, pair) stacked aT (bf16)
            aTg = pool_a.tile([PT, CHUNK], bf16, tag="aT", name=f"aT{c}_{g}")
            for h in (0, 1):
                dm = pool_dm.tile([DK, CHUNK], f32, tag="dm")
                nc.vector.reciprocal(dm[:, :], pso[h][DK:2 * DK, :])
                nc.vector.tensor_mul(aTg[h * DK:(h + 1) * DK, :],
                                     pso[h][0:DK, :], dm[:, :])
            return aTg

        def gen_outproj1(c, n, aTs):
            cw = slice(c * CHUNK, (c + 1) * CHUNK)
            psp = ps_mm.tile([PT, CHUNK], f32, tag="mm")
            for g in range(NG):
                nc.tensor.matmul(psp[:, :],
                                 wo_sb[:, g, n * PT:(n + 1) * PT],
                                 aTs[g][:, :],
                                 start=(g == 0), stop=(g == NG - 1))
                if g % 2 == 1:
                    yield 426
            ost = pool_os.tile([PT, CHUNK], bf16, tag="os")
            nc.vector.tensor_copy(ost[:, :], psp[:, :])
            nc.sync.dma_start(
                out=out_d.ap()[n * PT:(n + 1) * PT, cw],
                in_=ost[:, :])

        # Tile's scheduler turns emission order into each engine's static
        # execution order, so PE filler must be interleaved into the
        # ACT-paced attention stream at emission time.  Each attention
        # block evenly weaves the filler that the NEXT block depends on
        # (its q/k projection chunk, upcoming v tiles) as ~426ns generator
        # steps, and a credit pacer opportunistically weaves output
        # projections into whatever ACT-vs-PE deficit remains.
        opp = []             # opportunistic FIFO of generators
        credit = [0.0]

        def push_op(c, aTs):
            for n in range(D // PT):
                opp.append(gen_outproj1(c, n, aTs))

        def drain(gen):
            for _ in gen:
                pass

        def make_tick(mand, n_ticks, allow_opp=True):
            # mand: list of (gen, est_steps); weave so all mandatory steps
            # complete evenly across the block's n_ticks iterations
            total = sum(e for _, e in mand)
            state = {"i": 0, "done": 0}
            gens = [g for g, _ in mand]

            def tick(act_ns, pe_ns):
                state["i"] += 1
                credit[0] += act_ns - pe_ns
                target = total * state["i"] // n_ticks
                while state["done"] < target and gens:
                    try:
                        cost = next(gens[0])
                        state["done"] += 1
                        # PE-bound stretches don't borrow from future ACT
                        # slack: floor the credit instead of going deep red
                        credit[0] = max(credit[0] - cost, -426.0)
                    except StopIteration:
                        gens.pop(0)
                while allow_opp and opp and credit[0] >= 426:
                    try:
                        credit[0] -= next(opp[0])
                    except StopIteration:
                        opp.pop(0)

            def finish():
                for g in gens:
                    drain(g)
            return tick, finish

        # ramp: group-0 chunk-0 q/k and the first four v tiles run directly
        # (they gate the first attention block and are DMA-paced anyway)
        drain(gen_qk1(0, 0, 0, qTs[0][0], ramp=True))
        drain(gen_qk1(0, 0, CH, kTs[0][0], ramp=True))
        for t in range(4):
            drain(gen_v(t))

        aT_all = []
        vgens = {}
        for c in range(NCH):
            aTs = []
            for g in range(NG):
                mand = []
                if g == 0 and c >= 1:
                    # this section's top v tiles: needed only by its last 4
                    # key iterations, so they weave into the section itself,
                    # keeping the early (PE-bound) sections lighter
                    for t in range(4 * c, 4 * c + 4):
                        vgens[t] = gen_v(t)
                        mand.append((vgens[t], 5))
                if g < NG - 1:
                    mand += [(gen_qk1(g + 1, c, 0, qTs[g + 1][c]), 5),
                             (gen_qk1(g + 1, c, CH, kTs[g + 1][c]), 5)]
                elif c + 1 < NCH:
                    mand += [(gen_qk1(0, c + 1, 0, qTs[0][c + 1]), 5),
                             (gen_qk1(0, c + 1, CH, kTs[0][c + 1]), 5)]
                tick, finish = make_tick(mand, 2 * (c + 1) * 4)

                def need_v(kt):
                    gen = vgens.pop(kt, None)
                    if gen is not None:
                        drain(gen)
                aTs.append(emit_attn(c, g, tick=tick, need_v=need_v))
                finish()
            aT_all.append(aTs)
            push_op(c, aTs)
        for g in opp:
            drain(g)

    if split_waits:
        _split_multi_waits(nc)
    return nc


_build_cache = {}


def _get_program(Sc=S):
    key = Sc
    if key not in _build_cache:
        _build_cache[key] = build(Sc)
    return _build_cache[key]


def make_in_maps(x, w_q, w_k, w_v, w_o):
    """Host-side sharding: returns per-core input dicts.
    Core c: batch c % nb, head group c // nb (channels [512(c//nb), ...))."""
    Bc, Sc, Dc = x.shape
    xT = np.ascontiguousarray(x.transpose(0, 2, 1)).astype(np_bf16)
    jj, qq = np.meshgrid(np.arange(PT), np.arange(PT), indexing="ij")
    mask = np.where(jj <= qq, 0.0, NEG).astype(np.float32)
    scale = DK ** -0.5
    in_maps = []
    for c in range(NCORES):
        b, g = c % Bc, (c // Bc) % (D // CH)
        rows = slice(CH * g, CH * (g + 1))
        w3 = np.concatenate([(w_q[rows, :] * scale).T, w_k[rows, :].T,
                             w_v[rows, :].T], axis=1)
        w3b = np.ascontiguousarray(w3).astype(np_bf16)
        wr = np.concatenate([w3b[:, 0:PT], w3b[:, CH:CH + PT]], axis=1)
        in_maps.append({
            "xT": xT[b],
            "w3": w3b,
            "wr": np.ascontiguousarray(wr),
            "wo": np.ascontiguousarray(w_o[:, rows].T).astype(np_bf16),
            "mask": mask,
        })
    return in_maps


def run_on_hw(in_maps, Sc=S, trace=False):
    nc = _get_program(Sc)
    return bass_utils.run_bass_kernel_spmd(
        nc, in_maps, core_ids=list(range(NCORES)), trace=trace)


def kernel(x, w_q, w_k, w_v, w_o, b_o):
    x = np.asarray(x, dtype=np.float32)
    w_q = np.asarray(w_q, dtype=np.float32)
    w_k = np.asarray(w_k, dtype=np.float32)
    w_v = np.asarray(w_v, dtype=np.float32)
    w_o = np.asarray(w_o, dtype=np.float32)
    b_o = np.asarray(b_o, dtype=np.float32)
    Bc, Sc, Dc = x.shape
    in_maps = make_in_maps(x, w_q, w_k, w_v, w_o)
    res = run_on_hw(in_maps, Sc)
    out = np.zeros((Bc, Sc, Dc), dtype=np.float32)
    for c in range(NCORES):
        b = c % Bc
        out[b] += res.results[c]["outTp"].astype(np.float32).T
    out += b_o
    return out.astype(np.float32)


# revision 44
# speedup vs baseline: 1.9373x; 1.0185x over previous
"""Causal multi-head attention for Trainium2, 8-core batch x head-group parallel.

Problem: B=4, S=2048, D=1024, H=16 heads (dk=64), fp32 in/out.
    q = x @ w_q.T ; k = x @ w_k.T ; v = x @ w_v.T   (per-head split)
    out = softmax(causal(q k^T / 8)) v, concat heads, @ w_o.T + b_o

Sharding: core c owns batch (c % 4) and head group g = c // 4 (heads 8g..8g+7,
i.e. channels [512g, 512g+512)).  Each core computes q/k/v projections for its
512 channels over its single batch, runs causal attention for its 8 heads, and
produces the partial output projection outTp = (w_o[:, ch] a^T) of shape
[1024, S] in bf16; the host sums the two partials per batch (f32), transposes,
and adds b_o.

All device compute is bf16 (inputs converted host-side), fp32 PSUM accumulate:
rel err vs the fp32 reference lands ~1e-3, well under the 2e-2 gate, and bf16
halves DMA/SBUF footprint and DVE copy cost vs the fp32r baseline.

Per-core dataflow:
  - x pre-transposed on host to xT [D, S] so the contraction dim D lands on
    SBUF partitions; loaded once, as one batched DMA per 512-query chunk
    column (HWDGE issues serialize at ~625ns, so few big transfers win), and
    a small duplicated "ramp pack" carries the first attention block's q/k
    weight columns so the exp stream starts ~12us in.
  - qT, kT [512ch, S] via wT-stationary matmuls; channel tile g' holds head
    pair (2g', 2g'+1) stacked on partitions 0-63 / 64-127, and the two heads'
    dk=64 score matmuls are emitted back-to-back so they row-pack onto
    disjoint PE array halves (concurrent on hardware).
  - v computed in natural [tok, ch] layout (xT-chunk-stationary, wvT moving,
    N=512) -- no PE transposes needed -- and stored per token tile as
    [128, 8 heads x (64 v | 64 ones)]: the AV matmul out = [v|1]^T P then
    yields both the attention output (rows 0-63) and the softmax denominator
    replicated on rows 64-127 -- no partition reduction anywhere.
  - scores computed transposed (keys on partitions): sT = kT^T qT per 128-key
    tile, fp32 PSUM; softmax without max-subtraction (scores ~N(0,1), exp is
    safe in fp32), causal mask only on diagonal 128x128 blocks via a
    precomputed additive -1e9 mask; fully-masked column ranges are memset on
    gpsimd off the critical path, and the score/AV matmuls are narrowed to the
    valid query range on diagonal tiles.
  - normalization: reciprocal of the replicated denominator rows then one
    elementwise multiply, written straight into the stacked aT layout (bf16)
    that the output projection consumes.

Scheduling: Tile turns emission order into each engine's static execution
order, so the build interleaves instruction streams explicitly: the
ACT-paced attention stream (exp is the second-busiest engine at ~175us vs
PE ~226us) is woven at ~426ns granularity with generator-based PE filler
(later q/k projection chunks, upcoming v tiles, earlier chunks' output
projections), paced by a credit model of the per-iteration ACT-vs-PE
deficit.  PSUM banks: 2 proj/outproj + 4 score + 2 AV accumulators; the
4-deep score pool is what keeps the exp stream dense across the diagonal
(DVE-masked) iterations.  Simulated (TimelineSim cost model): ~251us/core,
~90% PE occupancy, vs ~357us for the fp32r head-sharded baseline.
"""

import numpy as np
import ml_dtypes

import concourse.bass as bass
import concourse.tile as tile
from concourse import mybir
from concourse import bass_utils

f32 = mybir.dt.float32
bf16 = mybir.dt.bfloat16
u32 = mybir.dt.uint32
AF = mybir.ActivationFunctionType
np_bf16 = ml_dtypes.bfloat16

B, S, D, H = 4, 2048, 1024, 16
DK = D // H            # 64
HC = H // 2            # 8 heads per core
CH = HC * DK           # 512 channels per core
NCORES = 8
PT = 128               # partition tile
CHUNK = 512            # query chunk
NEG = -1.0e9

_DMA_CLASSES = {"InstDMACopy", "InstTriggeredCopy", "InstDMATranspose", "InstDMAGatherAnt"}


def _split_multi_waits(nc):
    """This walrus build allows at most one sync-wait per TPB instruction;
    hoist extra waits onto single-wait NoOps on the same engine."""
    n = 0
    for f in nc.m.functions:
        for blk in f.blocks:
            new = []
            for inst in blk.instructions:
                si = inst.sync_info
                if si is not None and si.on_wait and len(si.on_wait) > 1:
                    ws = list(si.on_wait)
                    for w in ws[:-1]:
                        new.append(mybir.InstNoOp(
                            name=f"I-wfix-{n}", ins=[], outs=[], engine=inst.engine,
                            sync_info=mybir.SyncInfo(on_wait=[w], on_update=[])))
                        n += 1
                    inst.sync_info = mybir.SyncInfo(
                        on_wait=[ws[-1]], on_update=list(si.on_update))
                new.append(inst)
            blk.instructions = new
    return n


def build(Sc=S, split_waits=True, p_bufs=8, a_bufs=17, os_bufs=6, dm_bufs=3,
          mm_bufs=2, sc_bufs=4, av_bufs=2):
    """Build the per-core Bass program. Same program for all 8 cores; only
    the weight/x data differs per core."""
    from contextlib import ExitStack

    KT = D // PT           # 8 contraction tiles
    NCH = Sc // CHUNK      # query chunks
    NTT = Sc // PT         # token/key tiles
    NG = HC // 2           # 4 head-pair groups

    nc = bass.Bass("TRN2", target_bir_lowering=False, debug=False)

    xT_d = nc.dram_tensor("xT", [D, Sc], bf16, kind="ExternalInput")
    w3_d = nc.dram_tensor("w3", [D, 3 * CH], bf16, kind="ExternalInput")
    # ramp pack: group-0 q/k columns + all v columns, duplicated from w3 so
    # the first attention block's weights arrive in one small early transfer
    wr_d = nc.dram_tensor("wr", [D, 2 * PT], bf16, kind="ExternalInput")
    wo_d = nc.dram_tensor("wo", [CH, D], bf16, kind="ExternalInput")
    mask_d = nc.dram_tensor("mask", [PT, PT], f32, kind="ExternalInput")
    out_d = nc.dram_tensor("outTp", [D, Sc], bf16, kind="ExternalOutput")

    with tile.TileContext(nc) as tc, ExitStack() as ctx:
        singles = ctx.enter_context(tc.tile_pool(name="singles", bufs=1))
        pool_p = ctx.enter_context(tc.tile_pool(name="p", bufs=p_bufs))
        pool_a = ctx.enter_context(tc.tile_pool(name="a", bufs=a_bufs))
        pool_os = ctx.enter_context(tc.tile_pool(name="os", bufs=os_bufs))
        pool_dm = ctx.enter_context(tc.tile_pool(name="dm", bufs=dm_bufs))
        ps_mm = ctx.enter_context(tc.tile_pool(name="psmm", bufs=mm_bufs, space="PSUM"))
        ps_sc = ctx.enter_context(tc.tile_pool(name="pssc", bufs=sc_bufs, space="PSUM"))
        ps_av = ctx.enter_context(tc.tile_pool(name="psav", bufs=av_bufs, space="PSUM"))

        # ---- constants / persistent tensors ----
        # DMA order matters: x arrives in per-(kt, chunk) tiles and w3 in
        # per-kt qk/v halves so the first q/k projection chunk and the first
        # v tiles land ~7us in, instead of waiting for monolithic transfers;
        # later x chunks, wo, and mask queue up behind.
        # HWDGE issues serialize at ~625ns each, so batch the inputs into a
        # handful of transfers: x per chunk column, w3 in 2-kt slabs
        w3_sb = singles.tile([PT, KT, 3 * CH], bf16)
        wr_sb = singles.tile([PT, KT, 2 * PT], bf16)
        x_sb = [singles.tile([PT, KT, CHUNK], bf16, name=f"xc{c}")
                for c in range(NCH)]
        nc.sync.dma_start(
            out=wr_sb[:, :, :],
            in_=wr_d.ap().rearrange("(kt p) c -> p kt c", p=PT))
        nc.sync.dma_start(
            out=x_sb[0][:, :, :],
            in_=xT_d.ap()[:, 0:CHUNK].rearrange("(kt p) c -> p kt c", p=PT))
        mask_sb = singles.tile([PT, PT], f32)
        nc.sync.dma_start(out=mask_sb[:, :], in_=mask_d.ap())
        for kt in range(0, KT, 2):
            nc.sync.dma_start(
                out=w3_sb[:, kt:kt + 2, :],
                in_=w3_d.ap()[kt * PT:(kt + 2) * PT, :]
                .rearrange("(kt p) c -> p kt c", p=PT))
        for c in range(1, NCH):
            nc.sync.dma_start(
                out=x_sb[c][:, :, :],
                in_=xT_d.ap()[:, c * CHUNK:(c + 1) * CHUNK]
                .rearrange("(kt p) c -> p kt c", p=PT))
        wo_sb = singles.tile([PT, NG, D], bf16)
        nc.sync.dma_start(
            out=wo_sb[:, :, :],
            in_=wo_d.ap().rearrange("(g p) c -> p g c", p=PT))

        # per-chunk q/k tiles for fine-grained deps
        qTs = [[singles.tile([PT, CHUNK], bf16, name=f"q{g}_{c}")
                for c in range(NCH)] for g in range(NG)]
        kTs = [[singles.tile([PT, CHUNK], bf16, name=f"k{g}_{c}")
                for c in range(NCH)] for g in range(NG)]
        # v tiles: per token tile, 8 heads x [64 v-ch | 64 ones]
        v_sb = [singles.tile([PT, HC * 2 * DK], bf16, name=f"v{t}")
                for t in range(NTT)]
        for t in range(NTT):
            # pre-set the ones blocks (bf16 1.0 pairs = 0x3F803F80)
            nc.gpsimd.memset(
                v_sb[t][:, :].bitcast(u32)
                .rearrange("p (h x) -> p h x", x=DK)[:, :, DK // 2:],
                0x3F803F80)

        def gen_v(t):
            """Generator filler: v projection for token tile t, yielding
            after every 2 matmuls so the pacer can weave ~426ns steps."""
            tc_, tw = t // 4, slice((t % 4) * PT, (t % 4 + 1) * PT)
            psv = ps_mm.tile([PT, CH], f32, tag="mm")
            for kt in range(KT):
                nc.tensor.matmul(psv[:, :], x_sb[tc_][:, kt, tw],
                                 w3_sb[:, kt, 2 * CH:3 * CH],
                                 start=(kt == 0), stop=(kt == KT - 1))
                if kt % 2 == 1:
                    yield 426
            dst = v_sb[t][:, :].rearrange("p (h x) -> p h x", x=2 * DK)[:, :, 0:DK]
            src = psv[:, :].rearrange("p (h x) -> p h x", x=DK)
            nc.vector.tensor_copy(dst, src)

        def gen_qk1(g, c, w_off, dst, ramp=False):
            ps = ps_mm.tile([PT, CHUNK], f32, tag="mm")
            for kt in range(KT):
                w = (wr_sb[:, kt, (w_off // CH) * PT:(w_off // CH + 1) * PT]
                     if ramp else
                     w3_sb[:, kt, w_off + g * PT:w_off + (g + 1) * PT])
                nc.tensor.matmul(
                    ps[:, :], w, x_sb[c][:, kt, :],
                    start=(kt == 0), stop=(kt == KT - 1))
                if kt % 2 == 1:
                    yield 426
            nc.vector.tensor_copy(dst[:, :], ps[:, :])

        def emit_attn(c, g, tick=None, need_v=None):
            nkt = (c + 1) * (CHUNK // PT)      # causal: key tiles 0..nkt-1
            pso = {}
            for h in (0, 1):
                pso[h] = ps_av.tile([PT, CHUNK], f32, tag="av", name=f"pso{h}")
            for kt in range(nkt):
                if need_v is not None:
                    need_v(kt)
                i = kt - (c * (CHUNK // PT))
                q0 = max(i, 0) * PT   # first valid query col
                # the two heads' K=64 score matmuls are emitted back-to-back:
                # they sit on disjoint PE row halves (partitions 0-63/64-127,
                # auto tile_position), so real hardware runs them concurrently
                pss, P = {}, {}
                for h in (0, 1):
                    hp = slice(h * DK, (h + 1) * DK)
                    pss[h] = ps_sc.tile([PT, CHUNK], f32, tag="sc", name="pss")
                    nc.tensor.matmul(
                        pss[h][:, q0:CHUNK],
                        kTs[g][kt // 4][hp, (kt % 4) * PT:(kt % 4 + 1) * PT],
                        qTs[g][c][hp, q0:CHUNK],
                        start=True, stop=True)
                for h in (0, 1):
                    P[h] = pool_p.tile([PT, CHUNK], bf16, tag="P", name="P")
                    if i >= 0:
                        # diagonal key tile: triangular additive mask on the
                        # partial 128x128 block; fully-masked leading columns
                        # are never exp'd -- pre-zeroed on gpsimd.
                        if i > 0:
                            nc.gpsimd.memset(P[h][:, 0:q0].bitcast(u32), 0)
                        nc.vector.tensor_add(
                            pss[h][:, q0:q0 + PT], pss[h][:, q0:q0 + PT],
                            mask_sb[:, :])
                    nc.scalar.activation(out=P[h][:, q0:CHUNK],
                                         in_=pss[h][:, q0:CHUNK], func=AF.Exp)
                for h in (0, 1):
                    hh = 2 * g + h
                    nc.tensor.matmul(
                        pso[h][:, q0:CHUNK],
                        v_sb[kt][:, hh * 2 * DK:(hh + 1) * 2 * DK],
                        P[h][:, q0:CHUNK],
                        start=(kt == 0), stop=(kt == nkt - 1),
                        skip_group_check=True)
                    if tick is not None:
                        # ACT exp outpaces this iteration's PE work; let the
                        # pacer slot a low-priority PE filler group here.
                        tick(act_ns=62 + (CHUNK - q0) * 1.075,
                             pe_ns=2 * (CHUNK - q0) * 0.4167)
            # normalize into the per-(chunk, pair) stacked aT (bf16)
            aTg = pool_a.tile([PT, CHUNK], bf16, tag="aT", name=f"aT{c}_{g}")
            for h in (0, 1):
                dm = pool_dm.tile([DK, CHUNK], f32, tag="dm")
                nc.vector.reciprocal(dm[:, :], pso[h][DK:2 * DK, :])
                nc.vector.tensor_mul(aTg[h * DK:(h + 1) * DK, :],
                                     pso[h][0:DK, :], dm[:, :])
            return aTg

        def gen_outproj1(c, n, aTs):
            cw = slice(c * CHUNK, (c + 1) * CHUNK)
            psp = ps_mm.tile([PT, CHUNK], f32, tag="mm")
            for g in range(NG):
                nc.tensor.matmul(psp[:, :],
                                 wo_sb[:, g, n * PT:(n + 1) * PT],
                                 aTs[g][:, :],
                                 start=(g == 0), stop=(g == NG - 1))
                if g % 2 == 1:
                    yield 426
            ost = pool_os.tile([PT, CHUNK], bf16, tag="os")
            nc.vector.tensor_copy(ost[:, :], psp[:, :])
            nc.sync.dma_start(
                out=out_d.ap()[n * PT:(n + 1) * PT, cw],
                in_=ost[:, :])

        # Tile's scheduler turns emission order into each engine's static
        # execution order, so PE filler must be interleaved into the
        # ACT-paced attention stream at emission time.  Each attention
        # block evenly weaves the filler that the NEXT block depends on
        # (its q/k projection chunk, upcoming v tiles) as ~426ns generator
        # steps, and a credit pacer opportunistically weaves output
        # projections into whatever ACT-vs-PE deficit remains.
        opp = []             # opportunistic FIFO of generators
        credit = [0.0]

        def push_op(c, aTs):
            for n in range(D // PT):
                opp.append(gen_outproj1(c, n, aTs))

        def drain(gen):
            for _ in gen:
                pass

        def make_tick(mand, n_ticks, allow_opp=True):
            # mand: list of (gen, est_steps); weave so all mandatory steps
            # complete evenly across the block's n_ticks iterations
            total = sum(e for _, e in mand)
            state = {"i": 0, "done": 0}
            gens = [g for g, _ in mand]

            def tick(act_ns, pe_ns):
                state["i"] += 1
                credit[0] += act_ns - pe_ns
                target = total * state["i"] // n_ticks
                while state["done"] < target and gens:
                    try:
                        cost = next(gens[0])
                        state["done"] += 1
                        # PE-bound stretches don't borrow from future ACT
                        # slack: floor the credit instead of going deep red
                        credit[0] = max(credit[0] - cost, -426.0)
                    except StopIteration:
                        gens.pop(0)
                while allow_opp and opp and credit[0] >= 426:
                    try:
                        credit[0] -= next(opp[0])
                    except StopIteration:
                        opp.pop(0)

            def finish():
                for g in gens:
                    drain(g)
            return tick, finish

        # ramp: group-0 chunk-0 q/k and the first four v tiles run directly
        # (they gate the first attention block and are DMA-paced anyway)
        drain(gen_qk1(0, 0, 0, qTs[0][0], ramp=True))
        drain(gen_qk1(0, 0, CH, kTs[0][0], ramp=True))
        for t in range(4):
            drain(gen_v(t))

        aT_all = []
        vgens = {}
        for c in range(NCH):
            aTs = []
            for g in range(NG):
                mand = []
                if g == 0 and c >= 1:
                    # this section's top v tiles: needed only by its last 4
                    # key iterations, so they weave into the section itself,
                    # keeping the early (PE-bound) sections lighter
                    for t in range(4 * c, 4 * c + 4):
                        vgens[t] = gen_v(t)
                        mand.append((vgens[t], 5))
                if g < NG - 1:
                    mand += [(gen_qk1(g + 1, c, 0, qTs[g + 1][c]), 5),
                             (gen_qk1(g + 1, c, CH, kTs[g + 1][c]), 5)]
                elif c + 1 < NCH:
                    mand += [(gen_qk1(0, c + 1, 0, qTs[0][c + 1]), 5),
                             (gen_qk1(0, c + 1, CH, kTs[0][c + 1]), 5)]
                tick, finish = make_tick(mand, 2 * (c + 1) * 4)

                def need_v(kt):
                    gen = vgens.pop(kt, None)
                    if gen is not None:
                        drain(gen)
                aTs.append(emit_attn(c, g, tick=tick, need_v=need_v))
                finish()
            aT_all.append(aTs)
            push_op(c, aTs)
        for g in opp:
            drain(g)

    if split_waits:
        _split_multi_waits(nc)
    return nc


_build_cache = {}


def _get_program(Sc=S):
    key = Sc
    if key not in _build_cache:
        _build_cache[key] = build(Sc)
    return _build_cache[key]


def make_in_maps(x, w_q, w_k, w_v, w_o):
    """Host-side sharding: returns per-core input dicts.
    Core c: batch c % nb, head group c // nb (channels [512(c//nb), ...))."""
    Bc, Sc, Dc = x.shape
    xT = np.ascontiguousarray(x.transpose(0, 2, 1)).astype(np_bf16)
    jj, qq = np.meshgrid(np.arange(PT), np.arange(PT), indexing="ij")
    mask = np.where(jj <= qq, 0.0, NEG).astype(np.float32)
    scale = DK ** -0.5
    in_maps = []
    for c in range(NCORES):
        b, g = c % Bc, (c // Bc) % (D // CH)
        rows = slice(CH * g, CH * (g + 1))
        w3 = np.concatenate([(w_q[rows, :] * scale).T, w_k[rows, :].T,
                             w_v[rows, :].T], axis=1)
        w3b = np.ascontiguousarray(w3).astype(np_bf16)
        wr = np.concatenate([w3b[:, 0:PT], w3b[:, CH:CH + PT]], axis=1)
        in_maps.append({
            "xT": xT[b],
            "w3": w3b,
            "wr": np.ascontiguousarray(wr),
            "wo": np.ascontiguousarray(w_o[:, rows].T).astype(np_bf16),
            "mask": mask,
        })
    return in_maps


def run_on_hw(in_maps, Sc=S, trace=False):
    nc = _get_program(Sc)
    return bass_utils.run_bass_kernel_spmd(
        nc, in_maps, core_ids=list(range(NCORES)), trace=trace)


def kernel(x, w_q, w_k, w_v, w_o, b_o):
    x = np.asarray(x, dtype=np.float32)
    w_q = np.asarray(w_q, dtype=np.float32)
    w_k = np.asarray(w_k, dtype=np.float32)
    w_v = np.asarray(w_v, dtype=np.float32)
    w_o = np.asarray(w_o, dtype=np.float32)
    b_o = np.asarray(b_o, dtype=np.float32)
    Bc, Sc, Dc = x.shape
    in_maps = make_in_maps(x, w_q, w_k, w_v, w_o)
    res = run_on_hw(in_maps, Sc)
    out = np.zeros((Bc, Sc, Dc), dtype=np.float32)
    for c in range(NCORES):
        b = c % Bc
        out[b] += res.results[c]["outTp"].astype(np.float32).T
    out += b_o
    return out.astype(np.float32)


# revision 47
# speedup vs baseline: 2.6687x; 1.3775x over previous
"""Causal multi-head attention for Trainium2, 8-core batch x head-group parallel.

Problem: B=4, S=2048, D=1024, H=16 heads (dk=64), fp32 in/out.
    q = x @ w_q.T ; k = x @ w_k.T ; v = x @ w_v.T   (per-head split)
    out = softmax(causal(q k^T / 8)) v, concat heads, @ w_o.T + b_o

Sharding: core c owns batch (c % 4) and head group g = c // 4 (heads 8g..8g+7,
i.e. channels [512g, 512g+512)).  Each core computes q/k/v projections for its
512 channels over its single batch, runs causal attention for its 8 heads, and
produces the partial output projection outTp = (w_o[:, ch] a^T) of shape
[1024, S] in bf16; the host sums the two partials per batch (f32), transposes,
and adds b_o.

All device compute is bf16 (inputs converted host-side), fp32 PSUM accumulate:
rel err vs the fp32 reference lands ~1e-3, well under the 2e-2 gate, and bf16
halves DMA/SBUF footprint and DVE copy cost vs the fp32r baseline.

Per-core dataflow:
  - x pre-transposed on host to xT [D, S] so the contraction dim D lands on
    SBUF partitions; loaded once, as one batched DMA per 512-query chunk
    column (HWDGE issues serialize at ~625ns, so few big transfers win), and
    a small duplicated "ramp pack" carries the first attention block's q/k
    weight columns so the exp stream starts ~12us in.
  - qT, kT [512ch, S] via wT-stationary matmuls; channel tile g' holds head
    pair (2g', 2g'+1) stacked on partitions 0-63 / 64-127, and the two heads'
    dk=64 score matmuls are emitted back-to-back so they row-pack onto
    disjoint PE array halves (concurrent on hardware).
  - v computed in natural [tok, ch] layout (xT-chunk-stationary, wvT moving,
    N=512) -- no PE transposes needed -- and stored per token tile as
    [128, 8 heads x (64 v | 64 ones)]: the AV matmul out = [v|1]^T P then
    yields both the attention output (rows 0-63) and the softmax denominator
    replicated on rows 64-127 -- no partition reduction anywhere.
  - scores computed transposed (keys on partitions): sT = kT^T qT per 128-key
    tile, fp32 PSUM; softmax without max-subtraction (scores ~N(0,1), exp is
    safe in fp32), causal mask only on diagonal 128x128 blocks via a
    precomputed additive -1e9 mask; fully-masked column ranges are memset on
    gpsimd off the critical path, and the score/AV matmuls are narrowed to the
    valid query range on diagonal tiles.
  - normalization: reciprocal of the replicated denominator rows then one
    elementwise multiply, written straight into the stacked aT layout (bf16)
    that the output projection consumes.

Scheduling: Tile turns emission order into each engine's static execution
order, so the build interleaves instruction streams explicitly: the
ACT-paced attention stream (exp is the second-busiest engine at ~175us vs
PE ~226us) is woven at ~426ns granularity with generator-based PE filler
(later q/k projection chunks, upcoming v tiles, earlier chunks' output
projections), paced by a credit model of the per-iteration ACT-vs-PE
deficit.  PSUM banks: 2 proj/outproj + 4 score + 2 AV accumulators; the
4-deep score pool is what keeps the exp stream dense across the diagonal
(DVE-masked) iterations.  Simulated (TimelineSim cost model): ~251us/core,
~90% PE occupancy, vs ~357us for the fp32r head-sharded baseline.
"""

import numpy as np
import ml_dtypes

import concourse.bass as bass
import concourse.tile as tile
from concourse import mybir
from concourse import bass_utils

f32 = mybir.dt.float32
bf16 = mybir.dt.bfloat16
u32 = mybir.dt.uint32
AF = mybir.ActivationFunctionType
np_bf16 = ml_dtypes.bfloat16

B, S, D, H = 4, 2048, 1024, 16
DK = D // H            # 64
HC = H // 2            # 8 heads per core
CH = HC * DK           # 512 channels per core
NCORES = 8
PT = 128               # partition tile
CHUNK = 512            # query chunk
NEG = -1.0e9

_DMA_CLASSES = {"InstDMACopy", "InstTriggeredCopy", "InstDMATranspose", "InstDMAGatherAnt"}


def _split_multi_waits(nc):
    """This walrus build allows at most one sync-wait per TPB instruction;
    hoist extra waits onto single-wait NoOps on the same engine."""
    n = 0
    for f in nc.m.functions:
        for blk in f.blocks:
            new = []
            for inst in blk.instructions:
                si = inst.sync_info
                if si is not None and si.on_wait and len(si.on_wait) > 1:
                    ws = list(si.on_wait)
                    for w in ws[:-1]:
                        new.append(mybir.InstNoOp(
                            name=f"I-wfix-{n}", ins=[], outs=[], engine=inst.engine,
                            sync_info=mybir.SyncInfo(on_wait=[w], on_update=[])))
                        n += 1
                    inst.sync_info = mybir.SyncInfo(
                        on_wait=[ws[-1]], on_update=list(si.on_update))
                new.append(inst)
            blk.instructions = new
    return n


def build(Sc=S, split_waits=True, p_bufs=8, a_bufs=17, os_bufs=6, dm_bufs=3,
          mm_bufs=2, sc_bufs=4, av_bufs=2):
    """Build the per-core Bass program. Same program for all 8 cores; only
    the weight/x data differs per core."""
    from contextlib import ExitStack

    KT = D // PT           # 8 contraction tiles
    NCH = Sc // CHUNK      # query chunks
    NTT = Sc // PT         # token/key tiles
    NG = HC // 2           # 4 head-pair groups

    nc = bass.Bass("TRN2", target_bir_lowering=False, debug=False)

    xT_d = nc.dram_tensor("xT", [D, Sc], bf16, kind="ExternalInput")
    w3_d = nc.dram_tensor("w3", [D, 3 * CH], bf16, kind="ExternalInput")
    # ramp pack: group-0 q/k columns + all v columns, duplicated from w3 so
    # the first attention block's weights arrive in one small early transfer
    wr_d = nc.dram_tensor("wr", [D, 2 * PT], bf16, kind="ExternalInput")
    wo_d = nc.dram_tensor("wo", [CH, D], bf16, kind="ExternalInput")
    mask_d = nc.dram_tensor("mask", [PT, PT], f32, kind="ExternalInput")
    out_d = nc.dram_tensor("outTp", [D, Sc], bf16, kind="ExternalOutput")

    with tile.TileContext(nc) as tc, ExitStack() as ctx:
        singles = ctx.enter_context(tc.tile_pool(name="singles", bufs=1))
        pool_p = ctx.enter_context(tc.tile_pool(name="p", bufs=p_bufs))
        pool_a = ctx.enter_context(tc.tile_pool(name="a", bufs=a_bufs))
        pool_os = ctx.enter_context(tc.tile_pool(name="os", bufs=os_bufs))
        pool_dm = ctx.enter_context(tc.tile_pool(name="dm", bufs=dm_bufs))
        ps_mm = ctx.enter_context(tc.tile_pool(name="psmm", bufs=mm_bufs, space="PSUM"))
        ps_sc = ctx.enter_context(tc.tile_pool(name="pssc", bufs=sc_bufs, space="PSUM"))
        ps_av = ctx.enter_context(tc.tile_pool(name="psav", bufs=av_bufs, space="PSUM"))

        # ---- constants / persistent tensors ----
        # DMA order matters: x arrives in per-(kt, chunk) tiles and w3 in
        # per-kt qk/v halves so the first q/k projection chunk and the first
        # v tiles land ~7us in, instead of waiting for monolithic transfers;
        # later x chunks, wo, and mask queue up behind.
        # HWDGE issues serialize at ~625ns each, so batch the inputs into a
        # handful of transfers: x per chunk column, w3 in 2-kt slabs
        w3_sb = singles.tile([PT, KT, 3 * CH], bf16)
        wr_sb = singles.tile([PT, KT, 2 * PT], bf16)
        x_sb = [singles.tile([PT, KT, CHUNK], bf16, name=f"xc{c}")
                for c in range(NCH)]
        nc.sync.dma_start(
            out=wr_sb[:, :, :],
            in_=wr_d.ap().rearrange("(kt p) c -> p kt c", p=PT))
        nc.sync.dma_start(
            out=x_sb[0][:, :, :],
            in_=xT_d.ap()[:, 0:CHUNK].rearrange("(kt p) c -> p kt c", p=PT))
        mask_sb = singles.tile([PT, PT], f32)
        nc.sync.dma_start(out=mask_sb[:, :], in_=mask_d.ap())
        for kt in range(0, KT, 2):
            nc.sync.dma_start(
                out=w3_sb[:, kt:kt + 2, :],
                in_=w3_d.ap()[kt * PT:(kt + 2) * PT, :]
                .rearrange("(kt p) c -> p kt c", p=PT))
        for c in range(1, NCH):
            nc.sync.dma_start(
                out=x_sb[c][:, :, :],
                in_=xT_d.ap()[:, c * CHUNK:(c + 1) * CHUNK]
                .rearrange("(kt p) c -> p kt c", p=PT))
        wo_sb = singles.tile([PT, NG, D], bf16)
        nc.sync.dma_start(
            out=wo_sb[:, :, :],
            in_=wo_d.ap().rearrange("(g p) c -> p g c", p=PT))

        # per-chunk q/k tiles for fine-grained deps
        qTs = [[singles.tile([PT, CHUNK], bf16, name=f"q{g}_{c}")
                for c in range(NCH)] for g in range(NG)]
        kTs = [[singles.tile([PT, CHUNK], bf16, name=f"k{g}_{c}")
                for c in range(NCH)] for g in range(NG)]
        # v tiles: per token tile, 8 heads x [64 v-ch | 64 ones]
        v_sb = [singles.tile([PT, HC * 2 * DK], bf16, name=f"v{t}")
                for t in range(NTT)]
        for t in range(NTT):
            # pre-set the ones blocks (bf16 1.0 pairs = 0x3F803F80)
            nc.gpsimd.memset(
                v_sb[t][:, :].bitcast(u32)
                .rearrange("p (h x) -> p h x", x=DK)[:, :, DK // 2:],
                0x3F803F80)

        def gen_v(t):
            """Generator filler: v projection for token tile t, yielding
            after every 2 matmuls so the pacer can weave ~426ns steps."""
            tc_, tw = t // 4, slice((t % 4) * PT, (t % 4 + 1) * PT)
            psv = ps_mm.tile([PT, CH], f32, tag="mm")
            for kt in range(KT):
                nc.tensor.matmul(psv[:, :], x_sb[tc_][:, kt, tw],
                                 w3_sb[:, kt, 2 * CH:3 * CH],
                                 start=(kt == 0), stop=(kt == KT - 1))
                if kt % 2 == 1:
                    yield 426
            dst = v_sb[t][:, :].rearrange("p (h x) -> p h x", x=2 * DK)[:, :, 0:DK]
            src = psv[:, :].rearrange("p (h x) -> p h x", x=DK)
            nc.vector.tensor_copy(dst, src)

        def gen_qk1(g, c, w_off, dst, ramp=False):
            ps = ps_mm.tile([PT, CHUNK], f32, tag="mm")
            for kt in range(KT):
                w = (wr_sb[:, kt, (w_off // CH) * PT:(w_off // CH + 1) * PT]
                     if ramp else
                     w3_sb[:, kt, w_off + g * PT:w_off + (g + 1) * PT])
                nc.tensor.matmul(
                    ps[:, :], w, x_sb[c][:, kt, :],
                    start=(kt == 0), stop=(kt == KT - 1))
                if kt % 2 == 1:
                    yield 426
            nc.vector.tensor_copy(dst[:, :], ps[:, :])

        def emit_attn(c, g, tick=None, need_v=None):
            nkt = (c + 1) * (CHUNK // PT)      # causal: key tiles 0..nkt-1
            pso = {}
            for h in (0, 1):
                pso[h] = ps_av.tile([PT, CHUNK], f32, tag="av", name=f"pso{h}")
            for kt in range(nkt):
                if need_v is not None:
                    need_v(kt)
                i = kt - (c * (CHUNK // PT))
                q0 = max(i, 0) * PT   # first valid query col
                # the two heads' K=64 score matmuls are emitted back-to-back:
                # they sit on disjoint PE row halves (partitions 0-63/64-127,
                # auto tile_position), so real hardware runs them concurrently
                pss, P = {}, {}
                for h in (0, 1):
                    hp = slice(h * DK, (h + 1) * DK)
                    pss[h] = ps_sc.tile([PT, CHUNK], f32, tag="sc", name="pss")
                    nc.tensor.matmul(
                        pss[h][:, q0:CHUNK],
                        kTs[g][kt // 4][hp, (kt % 4) * PT:(kt % 4 + 1) * PT],
                        qTs[g][c][hp, q0:CHUNK],
                        start=True, stop=True)
                for h in (0, 1):
                    P[h] = pool_p.tile([PT, CHUNK], bf16, tag="P", name="P")
                    if i >= 0:
                        # diagonal key tile: triangular additive mask on the
                        # partial 128x128 block; fully-masked leading columns
                        # are never exp'd -- pre-zeroed on gpsimd.
                        if i > 0:
                            nc.gpsimd.memset(P[h][:, 0:q0].bitcast(u32), 0)
                        nc.vector.tensor_add(
                            pss[h][:, q0:q0 + PT], pss[h][:, q0:q0 + PT],
                            mask_sb[:, :])
                    nc.scalar.activation(out=P[h][:, q0:CHUNK],
                                         in_=pss[h][:, q0:CHUNK], func=AF.Exp)
                for h in (0, 1):
                    hh = 2 * g + h
                    nc.tensor.matmul(
                        pso[h][:, q0:CHUNK],
                        v_sb[kt][:, hh * 2 * DK:(hh + 1) * 2 * DK],
                        P[h][:, q0:CHUNK],
                        start=(kt == 0), stop=(kt == nkt - 1),
                        skip_group_check=True)
                    if tick is not None:
                        # ACT exp outpaces this iteration's PE work; let the
                        # pacer slot a low-priority PE filler group here.
                        tick(act_ns=62 + (CHUNK - q0) * 1.075,
                             pe_ns=2 * (CHUNK - q0) * 0.4167)
            # normalize into the per-(chunk, pair) stacked aT (bf16)
            aTg = pool_a.tile([PT, CHUNK], bf16, tag="aT", name=f"aT{c}_{g}")
            for h in (0, 1):
                dm = pool_dm.tile([DK, CHUNK], f32, tag="dm")
                nc.vector.reciprocal(dm[:, :], pso[h][DK:2 * DK, :])
                nc.vector.tensor_mul(aTg[h * DK:(h + 1) * DK, :],
                                     pso[h][0:DK, :], dm[:, :])
            return aTg

        def gen_outproj1(c, n, aTs):
            cw = slice(c * CHUNK, (c + 1) * CHUNK)
            psp = ps_mm.tile([PT, CHUNK], f32, tag="mm")
            for g in range(NG):
                nc.tensor.matmul(psp[:, :],
                                 wo_sb[:, g, n * PT:(n + 1) * PT],
                                 aTs[g][:, :],
                                 start=(g == 0), stop=(g == NG - 1))
                if g % 2 == 1:
                    yield 426
            ost = pool_os.tile([PT, CHUNK], bf16, tag="os")
            nc.vector.tensor_copy(ost[:, :], psp[:, :])
            nc.sync.dma_start(
                out=out_d.ap()[n * PT:(n + 1) * PT, cw],
                in_=ost[:, :])

        # Tile's scheduler turns emission order into each engine's static
        # execution order, so PE filler must be interleaved into the
        # ACT-paced attention stream at emission time.  Each attention
        # block evenly weaves the filler that the NEXT block depends on
        # (its q/k projection chunk, upcoming v tiles) as ~426ns generator
        # steps, and a credit pacer opportunistically weaves output
        # projections into whatever ACT-vs-PE deficit remains.
        opp = []             # opportunistic FIFO of generators
        credit = [0.0]

        def push_op(c, aTs):
            for n in range(D // PT):
                opp.append(gen_outproj1(c, n, aTs))

        def drain(gen):
            for _ in gen:
                pass

        def make_tick(mand, n_ticks, allow_opp=True):
            # mand: list of (gen, est_steps); weave so all mandatory steps
            # complete evenly across the block's n_ticks iterations
            total = sum(e for _, e in mand)
            state = {"i": 0, "done": 0}
            gens = [g for g, _ in mand]

            def tick(act_ns, pe_ns):
                state["i"] += 1
                credit[0] += act_ns - pe_ns
                target = total * state["i"] // n_ticks
                while state["done"] < target and gens:
                    try:
                        cost = next(gens[0])
                        state["done"] += 1
                        # PE-bound stretches don't borrow from future ACT
                        # slack: floor the credit instead of going deep red
                        credit[0] = max(credit[0] - cost, -426.0)
                    except StopIteration:
                        gens.pop(0)
                while allow_opp and opp and credit[0] >= 426:
                    try:
                        credit[0] -= next(opp[0])
                    except StopIteration:
                        opp.pop(0)

            def finish():
                for g in gens:
                    drain(g)
            return tick, finish

        # ramp: group-0 chunk-0 q/k and the first four v tiles run directly
        # (they gate the first attention block and are DMA-paced anyway)
        drain(gen_qk1(0, 0, 0, qTs[0][0], ramp=True))
        drain(gen_qk1(0, 0, CH, kTs[0][0], ramp=True))
        for t in range(4):
            drain(gen_v(t))

        aT_all = []
        vgens = {}
        for c in range(NCH):
            aTs = []
            for g in range(NG):
                mand = []
                if g == 0 and c >= 1:
                    # this section's top v tiles: needed only by its last 4
                    # key iterations, so they weave into the section itself,
                    # keeping the early (PE-bound) sections lighter
                    for t in range(4 * c, 4 * c + 4):
                        vgens[t] = gen_v(t)
                        mand.append((vgens[t], 5))
                if g < NG - 1:
                    mand += [(gen_qk1(g + 1, c, 0, qTs[g + 1][c]), 5),
                             (gen_qk1(g + 1, c, CH, kTs[g + 1][c]), 5)]
                elif c + 1 < NCH:
                    mand += [(gen_qk1(0, c + 1, 0, qTs[0][c + 1]), 5),
                             (gen_qk1(0, c + 1, CH, kTs[0][c + 1]), 5)]
                tick, finish = make_tick(mand, 2 * (c + 1) * 4)

                def need_v(kt):
                    gen = vgens.pop(kt, None)
                    if gen is not None:
                        drain(gen)
                aTs.append(emit_attn(c, g, tick=tick, need_v=need_v))
                finish()
            aT_all.append(aTs)
            push_op(c, aTs)
        for g in opp:
            drain(g)

    if split_waits:
        _split_multi_waits(nc)
    return nc


_build_cache = {}


def _get_program(Sc=S):
    key = Sc
    if key not in _build_cache:
        _build_cache[key] = build(Sc)
    return _build_cache[key]


def make_in_maps(x, w_q, w_k, w_v, w_o):
    """Host-side sharding: returns per-core input dicts.
    Core c: batch c % nb, head group c // nb (channels [512(c//nb), ...))."""
    Bc, Sc, Dc = x.shape
    xT = np.ascontiguousarray(x.transpose(0, 2, 1)).astype(np_bf16)
    jj, qq = np.meshgrid(np.arange(PT), np.arange(PT), indexing="ij")
    mask = np.where(jj <= qq, 0.0, NEG).astype(np.float32)
    scale = DK ** -0.5
    in_maps = []
    for c in range(NCORES):
        b, g = c % Bc, (c // Bc) % (D // CH)
        rows = slice(CH * g, CH * (g + 1))
        w3 = np.concatenate([(w_q[rows, :] * scale).T, w_k[rows, :].T,
                             w_v[rows, :].T], axis=1)
        w3b = np.ascontiguousarray(w3).astype(np_bf16)
        wr = np.concatenate([w3b[:, 0:PT], w3b[:, CH:CH + PT]], axis=1)
        in_maps.append({
            "xT": xT[b],
            "w3": w3b,
            "wr": np.ascontiguousarray(wr),
            "wo": np.ascontiguousarray(w_o[:, rows].T).astype(np_bf16),
            "mask": mask,
        })
    return in_maps


def run_on_hw(in_maps, Sc=S, trace=False):
    nc = _get_program(Sc)
    return bass_utils.run_bass_kernel_spmd(
        nc, in_maps, core_ids=list(range(NCORES)), trace=trace)


def kernel(x, w_q, w_k, w_v, w_o, b_o):
    x = np.asarray(x, dtype=np.float32)
    w_q = np.asarray(w_q, dtype=np.float32)
    w_k = np.asarray(w_k, dtype=np.float32)
    w_v = np.asarray(w_v, dtype=np.float32)
    w_o = np.asarray(w_o, dtype=np.float32)
    b_o = np.asarray(b_o, dtype=np.float32)
    Bc, Sc, Dc = x.shape
    in_maps = make_in_maps(x, w_q, w_k, w_v, w_o)
    res = run_on_hw(in_maps, Sc)
    out = np.zeros((Bc, Sc, Dc), dtype=np.float32)
    for c in range(NCORES):
        b = c % Bc
        out[b] += res.results[c]["outTp"].astype(np.float32).T
    out += b_o
    return out.astype(np.float32)
